# revision 11
# baseline (speedup 1.0000x reference)
"""Trainium2 Bass kernel for nn_ClassLoss (YOLO-style classification CE loss).

Strategy: the loss depends only on grid cells hit by valid target boxes
(<=50 cells/batch out of 4096). Each core handles 4 batches as 2 pair-columns
j in {0,1} over 100 partitions (2 batches x 50 boxes). Pipeline:
  1. one small DMA brings targets (+ per-partition batch offsets); a second
     (off critical path) brings the constant tables (class-index pattern,
     strictly-later-box mask, identity for PE transpose),
  2. box coords -> cell ids for both pair-columns in fused [100, 2k] ops,
  3. ONE merged indirect gather ([100,2] offsets -> [100, 2, 255] cell blocks),
  4. winner resolution (last valid write wins) via PE transpose + pairwise
     compare, overlapped with the gather,
  5. exp over all 480 class logits (one activation), per-(j,anchor) sums,
     log, and the label-logit sums via fused multiply-accumulate,
  6. DMA out [100, 6] partials: (sum_a lse, label_sum, winner) per column.
Host applies winner masks / per-batch mean (num / max(3*cnt,1)), sums across
cores and divides by the global batch size (the all-reduce + normalize of the
data-parallel sharding).
"""

import sys

sys.path.insert(0, "/opt/trn_rl_repo")

import numpy as np

import concourse.bass as bass
import concourse.tile as tile
from concourse import bacc, mybir
from concourse.bass_utils import run_bass_kernel_spmd

# Problem constants (hardcoded per harness contract).
B, A, H, W, NC_CLS, M = 32, 3, 64, 64, 80, 50
N_CORES = 8
B_CORE = B // N_CORES          # 4 batches per core
CELLS = H * W                  # 4096 cells per batch
ROWLEN = 3 * (5 + NC_CLS)      # 255 floats per cell (3 anchor rows x 85)
P2 = 2 * M                     # 100 partitions: 2 batches x 50 boxes
FP32 = mybir.dt.float32
I32 = mybir.dt.int32
Alu = mybir.AluOpType
Act = mybir.ActivationFunctionType

# const layout along free dim: cidx [0:255], ut2 doubled [255:455], ident [455:555]
CONST_F = ROWLEN + 2 * P2 + P2


def _host_consts():
    # cidx[*, col] = (col % 85) - 5: class id for class cols, negative (never
    # matches a class id >= 0) for the 5 box/objectness cols of each anchor.
    cidx = (np.arange(ROWLEN, dtype=np.float32) % 85.0) - 5.0
    cidx = np.broadcast_to(cidx, (P2, ROWLEN))
    # ut2[p, q] = 1 iff same 50-block and q%50 > p%50 (strictly-later box)
    blk = np.arange(P2) // M
    mi = np.arange(P2) % M
    ut2 = ((blk[:, None] == blk[None, :]) & (mi[None, :] > mi[:, None])).astype(
        np.float32
    )
    ident = np.eye(P2, dtype=np.float32)
    return np.ascontiguousarray(
        np.concatenate([cidx, ut2, ut2, ident], axis=1), dtype=np.float32
    )


_CONSTS = _host_consts()


def _host_tgt(targets_core):
    # [100, 12]: cols 0-4 = j0 (c,x,y,w,h), 5-9 = j1, 10-11 = cell offset of
    # batch (2j + p//50) in the flat [4*4096, 255] logits block.
    arr = np.zeros((P2, 12), dtype=np.float32)
    for j in range(2):
        for i in range(2):
            arr[i * M : (i + 1) * M, 5 * j : 5 * j + 5] = targets_core[2 * j + i]
            arr[i * M : (i + 1) * M, 10 + j] = (2 * j + i) * CELLS
    return arr


def _build_kernel_body(tc, x_ap, t_ap, c_ap, out_ap):
    nc = tc.nc
    from contextlib import ExitStack

    ctx = ExitStack()
    with ctx:
        consts = ctx.enter_context(tc.tile_pool(name="consts", bufs=1))
        work = ctx.enter_context(tc.tile_pool(name="work", bufs=2))
        gpool = ctx.enter_context(tc.tile_pool(name="gather", bufs=1))
        psum = ctx.enter_context(tc.tile_pool(name="psum", bufs=2, space="PSUM"))
        fpool = ctx.enter_context(tc.tile_pool(name="final", bufs=1))

        MAGIC = 8388608.0  # 2^23

        # ---- one manual act-table load of natural_log_exp_and_others (set 6:
        # has BOTH exp and ln) at stream head, so the compiler pass inserts no
        # per-switch ACT_TABLE_LOADs on the critical path ----
        nc.scalar.add_instruction(
            mybir.InstLoadActFuncSet(
                name="I-manual-atl", act_func_set_id=6, ins=[], outs=[]
            )
        )

        # ---- warmup SWDGE: absorb first-use overhead of the gather path ----
        woff = work.tile([2, 1], I32, tag="woff")
        nc.vector.memset(woff[:], 0)
        warm = work.tile([2, 12], FP32, tag="warm")
        nc.gpsimd.indirect_dma_start(
            out=warm[:],
            out_offset=None,
            in_=t_ap,
            in_offset=bass.IndirectOffsetOnAxis(ap=woff[:, :], axis=0),
        )

        # ---- input DMAs: targets first (gates everything), consts second ----
        tgt_t = consts.tile([P2, 12], FP32)
        nc.sync.dma_start(tgt_t[:], t_ap[:])
        const_t = consts.tile([P2, CONST_F], FP32)
        nc.sync.dma_start(const_t[:], c_ap[:])
        cidx_t = const_t[:, 0:ROWLEN]
        ut2d_t = const_t[:, ROWLEN : ROWLEN + 2 * P2]
        ident_t = const_t[:, ROWLEN + 2 * P2 : CONST_F]

        tj = tgt_t[:, 0:10].rearrange("p (j f) -> p j f", f=5)
        boff = tgt_t[:, 10:12]

        # ---- cell ids for both pair-columns: [100, 4] = (x0,y0,x1,y1)*64 ----
        # high_priority: keep the scheduler from slotting filler ops into this
        # chain — it gates the gathers, which gate everything.
        v4 = work.tile([P2, 4], FP32, tag="v4")
        iv4 = work.tile([P2, 4], I32, tag="iv4")
        fl4 = work.tile([P2, 4], FP32, tag="fl4")
        cellf = work.tile([P2, 2], FP32, tag="cellf")
        celli = work.tile([P2, 2], I32, tag="celli")
        with tc.high_priority():
            nc.vector.tensor_scalar(
                v4[:].rearrange("p (j f) -> p j f", f=2), tj[:, :, 1:3], 64.0,
                None, op0=Alu.mult,
            )
            # floor via round-to-nearest on the int32 write: rint(v-0.5) ==
            # trunc(v) for v >= 0 not an exact odd integer (valid coords are
            # never exact integers; padding rows give v=0, also correct)
            nc.vector.tensor_scalar(iv4[:], v4[:], -0.5, None, op0=Alu.add)
            nc.vector.tensor_scalar(fl4[:], iv4[:], 0, None, op0=Alu.add)
            flv = fl4[:].rearrange("p (j f) -> p j f", f=2)
            # cell = y*64 + x; celli = cell + batch offset (int32 gather idx)
            nc.vector.scalar_tensor_tensor(
                cellf[:], flv[:, :, 1], 64.0, flv[:, :, 0], op0=Alu.mult,
                op1=Alu.add,
            )
            nc.vector.tensor_tensor(celli[:], cellf[:], boff, op=Alu.add)

        # ---- two indirect gathers (HW: one offset per partition), [100,255]
        # cell blocks each; the j1 gather overlaps the j0 CE tail ----
        graw = gpool.tile([P2, 2 * ROWLEN], FP32, tag="graw")
        for j in range(2):
            nc.gpsimd.indirect_dma_start(
                out=graw[:, j * ROWLEN : (j + 1) * ROWLEN],
                out_offset=None,
                in_=x_ap,
                in_offset=bass.IndirectOffsetOnAxis(ap=celli[:, j : j + 1], axis=0),
            )

        # ---- winner resolution (last valid write wins), overlaps gather ----
        val2 = work.tile([P2, 2], FP32, tag="val2")
        nc.vector.tensor_reduce(
            val2[:], tj, axis=mybir.AxisListType.X, op=Alu.add,
            apply_absolute_value=True,
        )
        valid2 = work.tile([P2, 2], FP32, tag="valid2")
        nc.vector.tensor_scalar(valid2[:], val2[:], 0.0, None, op0=Alu.is_gt)
        # key = valid ? cell : -1
        key2 = work.tile([P2, 2], FP32, tag="key2")
        nc.vector.scalar_tensor_tensor(
            key2[:], cellf[:], 1.0, valid2[:], op0=Alu.add, op1=Alu.mult
        )
        nc.vector.tensor_scalar(key2[:], key2[:], -1.0, None, op0=Alu.add)

        # one-hot of the class id over the 255 row cols (overlaps gather)
        ohc = gpool.tile([P2, 2 * ROWLEN], FP32, tag="ohc")
        for j in range(2):
            nc.vector.tensor_scalar(
                ohc[:, j * ROWLEN : (j + 1) * ROWLEN], cidx_t,
                tgt_t[:, 5 * j : 5 * j + 1], None, op0=Alu.is_equal,
            )

        sameD = work.tile([P2, 2 * P2], FP32, tag="sameD")
        for j in range(2):
            qT = psum.tile([P2, P2], FP32, tag=f"qT{j}", space="PSUM")
            nc.tensor.transpose(
                qT[:], key2[:, j : j + 1].to_broadcast([P2, P2]), ident_t
            )
            nc.vector.tensor_scalar(
                sameD[:, j * P2 : (j + 1) * P2], qT[:], key2[:, j : j + 1],
                None, op0=Alu.is_equal,
            )
        nc.vector.tensor_tensor(sameD[:], sameD[:], ut2d_t, op=Alu.mult)
        coll2 = work.tile([P2, 2], FP32, tag="coll2")
        nc.vector.tensor_reduce(
            coll2[:], sameD[:].rearrange("p (j q) -> p j q", q=P2),
            axis=mybir.AxisListType.X, op=Alu.add,
        )

        # ---- output partials tile: [100, 6] = (lse_sum x2, label_sum x2,
        # winner x2) ----
        res = fpool.tile([P2, 6], FP32)
        # winner = valid & (no strictly-later box hit the same cell)
        nc.vector.scalar_tensor_tensor(
            res[:, 4:6], coll2[:], 0.0, valid2[:], op0=Alu.is_equal, op1=Alu.mult
        )

        # ---- CE tail, j0 pipelined with the j1 gather; one merged Ln.
        # Per-anchor Exp with accum_out: sum(exp) lands in se6 straight from
        # the Scalar engine, no DVE reduce on the critical path. ----
        ex = gpool.tile([P2, 2 * 3 * NC_CLS], FP32, tag="ex")
        se6 = work.tile([P2, 6], FP32, tag="se6")
        lse6 = work.tile([P2, 6], FP32, tag="lse6")
        scrap = work.tile([P2, ROWLEN], FP32, tag="scrap")
        for j in range(2):
            for a in range(3):
                colv = graw[:, j * ROWLEN + a * 85 + 5 : j * ROWLEN + (a + 1) * 85]
                exa = ex[:, (3 * j + a) * NC_CLS : (3 * j + a + 1) * NC_CLS]
                nc.scalar.activation(
                    exa, colv, Act.Exp,
                    accum_out=se6[:, 3 * j + a : 3 * j + a + 1],
                )
            # label-logit sum: fused multiply + accumulate
            nc.vector.scalar_tensor_tensor(
                scrap[:], graw[:, j * ROWLEN : (j + 1) * ROWLEN], 1.0,
                ohc[:, j * ROWLEN : (j + 1) * ROWLEN],
                op0=Alu.mult, op1=Alu.mult,
                accum_out=res[:, 2 + j : 3 + j],
            )
        nc.scalar.activation(lse6[:], se6[:], Act.Ln)
        nc.vector.tensor_reduce(
            res[:, 0:2], lse6[:].rearrange("p (j a) -> p j a", a=3),
            axis=mybir.AxisListType.X, op=Alu.add,
        )

        nc.sync.dma_start(out_ap[:], res[:])


_CACHE = {}


def _get_compiled():
    if "nc" in _CACHE:
        return _CACHE["nc"]
    nc = bacc.Bacc(
        "TRN2",
        target_bir_lowering=False,
        debug=False,
        enable_asserts=False,
        num_devices=N_CORES,
    )
    x = nc.dram_tensor("xflat", [B_CORE * CELLS, ROWLEN], FP32, kind="ExternalInput")
    t = nc.dram_tensor("tgt", [P2, 12], FP32, kind="ExternalInput")
    c = nc.dram_tensor("cst", [P2, CONST_F], FP32, kind="ExternalInput")
    out = nc.dram_tensor("resout", [P2, 6], FP32, kind="ExternalOutput")

    with tile.TileContext(nc) as tc:
        _build_kernel_body(tc, x.ap(), t.ap(), c.ap(), out.ap())
    nc.compile()
    _CACHE["nc"] = nc
    return nc


def _finish(res_list):
    """Host: winner-masked per-batch mean, then mean over global batch."""
    total = 0.0
    for r in res_list:
        r = np.asarray(r, dtype=np.float64)  # [100, 6]
        d = r[:, 0:2] - r[:, 2:4]
        win = r[:, 4:6]
        for j in range(2):
            for i in range(2):
                sel = slice(i * M, (i + 1) * M)
                num = float(np.sum(win[sel, j] * d[sel, j]))
                cnt = float(np.sum(win[sel, j]))
                total += num / max(3.0 * cnt, 1.0)
    return total / B


def _run(output, targets, trace=False):
    nc = _get_compiled()
    output = np.ascontiguousarray(output, dtype=np.float32)
    targets = np.ascontiguousarray(targets, dtype=np.float32)
    in_maps = []
    for k in range(N_CORES):
        in_maps.append(
            {
                "xflat": output[k * B_CORE : (k + 1) * B_CORE].reshape(
                    B_CORE * CELLS, ROWLEN
                ),
                "tgt": _host_tgt(targets[k * B_CORE : (k + 1) * B_CORE]),
                "cst": _CONSTS,
            }
        )
    res = run_bass_kernel_spmd(nc, in_maps, core_ids=list(range(N_CORES)), trace=trace)
    total = _finish([r["resout"] for r in res.results])
    return np.float32(total), res


def kernel(output, targets):
    val, _ = _run(output, targets)
    return np.asarray(val, dtype=np.float32)
